# revision 1
# baseline (speedup 1.0000x reference)
"""Trainium2 Bass kernel for GQA attention forward (B=2, S=2048, D=2048,
16 q-heads / 4 kv-heads, head_dim=128, RoPE, causal).

Sharding: 8 cores = 2 (batch) x 4 (kv-head groups). Each core computes its
batch's attention for one kv-head group (4 q-heads + 1 kv head) and a
row-parallel partial of the output projection; the host sums the 4 partials
per batch.

Matmul operands are bf16 (1 cycle/row on PE) with fp32 PSUM accumulation;
the softmax denominator path runs in fp32/f32r to avoid bf16 rounding of
the normalization.
"""

import sys

if "/opt/trn_rl_repo" not in sys.path:
    sys.path.insert(0, "/opt/trn_rl_repo")

import numpy as np
import ml_dtypes

import concourse.bass as bass
import concourse.tile as tile
from concourse import mybir

F32 = mybir.dt.float32
F32R = mybir.dt.float32r
BF16 = mybir.dt.bfloat16

# Full-problem constants (per reference).
B, S, DIM = 2, 2048, 2048
N_HEADS, N_KV_HEADS, HEAD_DIM = 16, 4, 128
N_GROUPS = N_KV_HEADS          # tensor-parallel groups
HQ = N_HEADS // N_KV_HEADS     # q heads per group
NEG = -1e30


def build_attention_core(nc, S=S, D=DIM, HQ=HQ, HD=HEAD_DIM, CHUNK=512):
    """Emit the per-core attention program into `nc` (Tile framework).

    Inputs (ExternalInput dram tensors):
      x      [S, D]  bf16   activations for this core's batch
      wqT    [D, HQ*HD] bf16  q projection, transposed, RoPE-permuted rows
      wkvT   [D, 2*HD] bf16   [wk^T | wv^T] (wk RoPE-permuted)
      woT    [HQ*HD, D] bf16  output projection slice, transposed
      t1,t2  [S, HD] f32      RoPE tables (permuted-half layout)
      masks  [CHUNK//128, 128, CHUNK] f32 additive causal masks
      ident  [128, 128] bf16  identity for PE transposes
      ones_col [128,1] bf16 / ones_row [1,128] f32r
    Output:
      out_partial [S, D] f32
    """
    n_st = S // 128        # s tiles
    n_dt = D // 128        # d tiles
    n_ch = S // CHUNK      # q chunks
    kpc = CHUNK // 128     # k-tiles per chunk
    n_dc = D // CHUNK      # d chunks (phase C)
    IQ = HQ * HD

    x_d = nc.dram_tensor("xT", [128, D // 128, S], BF16, kind="ExternalInput")
    wqT_d = nc.dram_tensor("wqT", [128, D // 128, IQ], BF16, kind="ExternalInput")
    wkvT_d = nc.dram_tensor("wkvT", [128, D // 128, 2 * HD], BF16, kind="ExternalInput")
    woT_d = nc.dram_tensor("woT", [128, IQ // 128, D], BF16, kind="ExternalInput")
    t1_d = nc.dram_tensor("t1", [128, S // 128, HD], F32, kind="ExternalInput")
    t2_d = nc.dram_tensor("t2", [128, S // 128, HD], F32, kind="ExternalInput")
    masks_d = nc.dram_tensor("masks", [128, kpc, CHUNK], F32, kind="ExternalInput")
    ident_d = nc.dram_tensor("ident", [128, 128], BF16, kind="ExternalInput")
    onesc_d = nc.dram_tensor("ones_col", [128, 1], BF16, kind="ExternalInput")
    onesr_d = nc.dram_tensor("ones_row", [1, 128], F32R, kind="ExternalInput")
    out_d = nc.dram_tensor("out_partial", [S, D], F32, kind="ExternalOutput")

    scale = float(HD) ** -0.5

    with tile.TileContext(nc) as tc:
        with (
            # tensors persistent across phases
            tc.tile_pool(name="persist", bufs=1) as persist,
            tc.tile_pool(name="constB", bufs=1) as constB,
        ):
            qT_sb = persist.tile([128, HQ, S], BF16)    # [e, h, s]
            kT_sb = persist.tile([128, S], BF16)        # [e, s]
            v_sb = persist.tile([128, n_st, HD], BF16)  # [s_in_tile, s_tile, e]
            oT_sb = persist.tile([128, HQ, S], BF16)    # [e, h, s]

            # ---------------- Phase A: projections + RoPE -------------------
            with (
                tc.tile_pool(name="weightsA", bufs=1) as weightsA,
                tc.tile_pool(name="xt", bufs=1) as xt_pool,
                tc.tile_pool(name="rope", bufs=4) as rope_pool,
                tc.tile_pool(name="ps_t", bufs=3, space="PSUM") as pst_pool,
                tc.tile_pool(name="ps_q", bufs=2, space="PSUM") as psq_pool,
                tc.tile_pool(name="ps_kv", bufs=2, space="PSUM") as pskv_pool,
            ):
                gq = n_dt // 4
                wq_g = []
                wkv_g = []
                for g in range(4):
                    wqg = weightsA.tile([128, gq, IQ], BF16, tag=f"wq{g}",
                                        name=f"wq{g}")
                    nc.scalar.dma_start(
                        out=wqg, in_=wqT_d[:, g * gq:(g + 1) * gq, :]
                    )
                    wq_g.append(wqg)
                    wkvg = weightsA.tile([128, gq, 2 * HD], BF16, tag=f"wkv{g}",
                                         name=f"wkv{g}")
                    nc.scalar.dma_start(
                        out=wkvg, in_=wkvT_d[:, g * gq:(g + 1) * gq, :]
                    )
                    wkv_g.append(wkvg)
                ident = weightsA.tile([128, 128], BF16)
                nc.scalar.dma_start(out=ident, in_=ident_d[:])
                t1_sb = weightsA.tile([128, n_st, HD], F32)
                nc.scalar.dma_start(
                    out=t1_sb, in_=t1_d[:]
                )
                t2_sb = weightsA.tile([128, n_st, HD], F32)
                nc.scalar.dma_start(
                    out=t2_sb, in_=t2_d[:]
                )

                # pre-transposed activations: 2 d-tiles per DMA, sync queue
                xpair = []
                for g2 in range(n_dt // 2):
                    xt_t = xt_pool.tile([128, 2, S], BF16, tag=f"xt{g2}")
                    nc.sync.dma_start(out=xt_t, in_=x_d[:, g2 * 2:g2 * 2 + 2, :])
                    xpair.append(xt_t)
                xT = [xpair[dt_ // 2][:, dt_ % 2, :] for dt_ in range(n_dt)]

                # phase-B constants loaded early so the first diagonal
                # mask-add never waits
                masks_sb = constB.tile([128, kpc, CHUNK], F32)
                nc.sync.dma_start(out=masks_sb, in_=masks_d[:])
                ones_col = constB.tile([128, 1], BF16)
                nc.sync.dma_start(out=ones_col, in_=onesc_d[:])
                ones_row = constB.tile([1, 128], F32R)
                nc.sync.dma_start(out=ones_row, in_=onesr_d[:])

                rope_pending = None

                def emit_transposes(rp):
                    q_rot_, k_rot_, sl_ = rp
                    for h in range(HQ):
                        ps_tq = pst_pool.tile([128, 128], BF16, tag="ps_t")
                        nc.tensor.transpose(
                            ps_tq, q_rot_[:, h * HD:(h + 1) * HD], ident
                        )
                        nc.vector.tensor_copy(qT_sb[:, h, sl_], ps_tq)
                    ps_tk = pst_pool.tile([128, 128], BF16, tag="ps_t")
                    nc.tensor.transpose(ps_tk, k_rot_, ident)
                    nc.vector.tensor_copy(kT_sb[:, sl_], ps_tk)

                for st in range(n_st):
                    ps_q = psq_pool.tile([128, IQ], F32)
                    ps_kv = pskv_pool.tile([128, 2 * HD], F32)
                    st_sl = slice(st * 128, (st + 1) * 128)
                    for dt_ in range(n_dt):
                        nc.tensor.matmul(
                            ps_kv, xT[dt_][:, st_sl], wkv_g[dt_ // gq][:, dt_ % gq, :],
                            start=(dt_ == 0), stop=(dt_ == n_dt - 1),
                        )
                        nc.tensor.matmul(
                            ps_q, xT[dt_][:, st_sl], wq_g[dt_ // gq][:, dt_ % gq, :],
                            start=(dt_ == 0), stop=(dt_ == n_dt - 1),
                        )
                    # previous s-tile's PE transposes: emitted here so PE
                    # never waits on the DVE RoPE chain
                    if rope_pending is not None:
                        emit_transposes(rope_pending)
                        rope_pending = None

                    # RoPE on all q heads at once (tables broadcast
                    # across heads via zero-stride AP)
                    t1s = t1_sb[:, st, :]
                    t2s = t2_sb[:, st, :]
                    t1b = bass.AP(tensor=t1s.tensor, offset=t1s.offset,
                                  ap=[t1s.ap[0], [0, HQ], t1s.ap[1]])
                    t2b = bass.AP(tensor=t2s.tensor, offset=t2s.offset,
                                  ap=[t2s.ap[0], [0, HQ], t2s.ap[1]])
                    ps_qv = ps_q.rearrange("p (h e) -> p h e", h=HQ)
                    t1m = rope_pool.tile([128, HQ, HD], F32, tag="t1m")
                    nc.vector.tensor_mul(t1m, ps_qv, t1b)
                    t2m = rope_pool.tile([128, HQ, HD], F32, tag="t2m")
                    nc.vector.tensor_mul(
                        t2m[:, :, 0:64], ps_qv[:, :, 64:128], t2b[:, :, 0:64]
                    )
                    nc.vector.tensor_mul(
                        t2m[:, :, 64:128], ps_qv[:, :, 0:64], t2b[:, :, 64:128]
                    )
                    q_rot = rope_pool.tile([128, HQ * HD], BF16, tag="qrot")
                    nc.vector.tensor_add(
                        q_rot.rearrange("p (h e) -> p h e", h=HQ), t1m, t2m
                    )
                    # RoPE on k
                    t1mk = rope_pool.tile([128, HD], F32, tag="t1mk")
                    nc.vector.tensor_mul(t1mk, ps_kv[:, 0:HD], t1_sb[:, st, :])
                    t2mk = rope_pool.tile([128, HD], F32, tag="t2mk")
                    nc.vector.tensor_mul(
                        t2mk[:, 0:64], ps_kv[:, 64:128], t2_sb[:, st, 0:64]
                    )
                    nc.vector.tensor_mul(
                        t2mk[:, 64:128], ps_kv[:, 0:64], t2_sb[:, st, 64:128]
                    )
                    k_rot = rope_pool.tile([128, HD], BF16, tag="krot")
                    nc.vector.tensor_add(k_rot, t1mk, t2mk)
                    rope_pending = (q_rot, k_rot, st_sl)

                    # v: straight copy (natural [s, e] layout), cast to bf16
                    nc.scalar.copy(v_sb[:, st, :], ps_kv[:, HD:2 * HD])
                emit_transposes(rope_pending)

            # ---------------- Phases B+C shared: woT ------------------------
            with tc.tile_pool(name="weightsC", bufs=1) as weightsC:
                woT_sb = weightsC.tile([128, IQ // 128, D], BF16)
                nc.sync.dma_start(
                    out=woT_sb, in_=woT_d[:]
                )

                # ---------------- Phase B: attention ------------------------
                with (
                    tc.tile_pool(name="expt", bufs=4) as expt_pool,
                    tc.tile_pool(name="maskbuf", bufs=3) as mask_pool,
                    tc.tile_pool(name="sums", bufs=2) as sums_pool,
                    tc.tile_pool(name="recip", bufs=2) as rec_pool,
                    tc.tile_pool(name="ps_s", bufs=3, space="PSUM") as pss_pool,
                    tc.tile_pool(name="ps_o", bufs=2, space="PSUM") as pso_pool,
                    tc.tile_pool(name="ps_sum", bufs=2, space="PSUM") as pssum_pool,
                    tc.tile_pool(name="ps_b", bufs=1, space="PSUM") as psb_pool,
                ):
                    norm_pending = [None]

                    def emit_norm():
                        ps_o_, ps_sum_, h_, c_ = norm_pending[0]
                        norm_pending[0] = None
                        sums_sb = sums_pool.tile([1, CHUNK], F32R, tag="sums")
                        with nc.allow_low_precision(reason="f32r denom"):
                            nc.vector.tensor_copy(sums_sb, ps_sum_)
                        ps_b = psb_pool.tile([128, CHUNK], F32)
                        nc.tensor.matmul(
                            ps_b, ones_row, sums_sb, start=True, stop=True,
                        )
                        recip = rec_pool.tile([128, CHUNK], F32)
                        nc.vector.reciprocal_approx_fast(recip, ps_b)
                        nc.vector.tensor_mul(
                            oT_sb[:, h_, c_ * CHUNK:(c_ + 1) * CHUNK],
                            ps_o_, recip,
                        )

                    for h in range(HQ):
                        for c in range(n_ch):
                            ps_o = pso_pool.tile([128, CHUNK], F32)
                            ps_sum = pssum_pool.tile([1, CHUNK], F32)
                            n_kj = (c + 1) * kpc
                            c_sl = slice(c * CHUNK, (c + 1) * CHUNK)
                            pending = []

                            def flush_one():
                                pe, pj, poff = pending.pop(0)
                                nc.tensor.matmul(
                                    ps_o[:, poff:], v_sb[:, pj, :], pe,
                                    start=(pj == 0), stop=(pj == n_kj - 1),
                                )
                                nc.tensor.matmul(
                                    ps_sum[:, poff:], ones_col, pe,
                                    start=(pj == 0), stop=(pj == n_kj - 1),
                                )

                            for kj in range(n_kj):
                                # columns left of the diagonal block are fully
                                # masked: skip them (q >= kj*128 only)
                                off = max(0, (kj - c * kpc)) * 128
                                w = CHUNK - off
                                ps_s = pss_pool.tile([128, CHUNK], F32, tag="ps_s")
                                nc.tensor.matmul(
                                    ps_s[:, 0:w],
                                    kT_sb[:, kj * 128:(kj + 1) * 128],
                                    qT_sb[:, h, c * CHUNK + off:(c + 1) * CHUNK],
                                    start=True, stop=True,
                                )
                                if kj >= c * kpc:  # diagonal chunk: causal mask
                                    msk = mask_pool.tile([128, CHUNK], F32, tag="msk")
                                    nc.vector.tensor_add(
                                        msk[:, 0:w], ps_s[:, 0:w],
                                        masks_sb[:, kj % kpc, off:],
                                    )
                                    exp_in = msk
                                else:
                                    exp_in = ps_s
                                expT = expt_pool.tile([128, CHUNK], BF16, tag="expT")
                                nc.scalar.activation(
                                    expT[:, 0:w], exp_in[:, 0:w],
                                    mybir.ActivationFunctionType.Exp,
                                    scale=scale,
                                )
                                pending.append((expT[:, 0:w], kj, off))
                                if kj == 1 and norm_pending[0] is not None:
                                    emit_norm()
                                if len(pending) > 2:
                                    flush_one()
                            while pending:
                                flush_one()
                            norm_pending[0] = (ps_o, ps_sum, h, c)

                    emit_norm()

                # ---------------- Phase C: output projection ----------------
                with (
                    tc.tile_pool(name="outsb", bufs=2) as outsb_pool,
                    tc.tile_pool(name="ps_d", bufs=4, space="PSUM") as psd_pool,
                ):
                    for st in range(n_st):
                        out_sb = outsb_pool.tile([128, D], F32)
                        for dc in range(n_dc):
                            ps_d = psd_pool.tile([128, CHUNK], F32)
                            for it in range(HQ):
                                nc.tensor.matmul(
                                    ps_d,
                                    oT_sb[:, it, st * 128:(st + 1) * 128],
                                    woT_sb[:, it, dc * CHUNK:(dc + 1) * CHUNK],
                                    start=(it == 0), stop=(it == HQ - 1),
                                )
                            nc.scalar.copy(
                                out_sb[:, dc * CHUNK:(dc + 1) * CHUNK], ps_d
                            )
                        nc.sync.dma_start(
                            out=out_d[st * 128:(st + 1) * 128, :], in_=out_sb
                        )

    return nc


# ---------------------------------------------------------------------------
# Host-side prep


_ROPE_PERM = np.concatenate([np.arange(0, HEAD_DIM, 2), np.arange(1, HEAD_DIM, 2)])


def _prep_tables(freq_cis, S_=S, HD_=HEAD_DIM):
    """RoPE tables in permuted-half layout: rot = q*t1 + swap(q)*t2."""
    fc = np.asarray(freq_cis, dtype=np.float32)
    A = fc[:, :, 0, 0]
    Bm = fc[:, :, 0, 1]
    C = fc[:, :, 1, 0]
    Dm = fc[:, :, 1, 1]
    t1 = np.concatenate([A, Dm], axis=1).astype(np.float32)  # [S, HD]
    t2 = np.concatenate([Bm, C], axis=1).astype(np.float32)
    return np.ascontiguousarray(t1), np.ascontiguousarray(t2)


def _prep_masks(chunk=512):
    kpc = chunk // 128
    masks = np.zeros((kpc, 128, chunk), dtype=np.float32)
    q = np.arange(chunk)[None, :]
    p = np.arange(128)[:, None]
    for j in range(kpc):
        masks[j] = np.where(q >= j * 128 + p, 0.0, NEG).astype(np.float32)
    return masks


def _perm_head_rows(w):
    """Permute rows within each 128-row head block: evens first, odds second."""
    nh = w.shape[0] // HEAD_DIM
    return np.ascontiguousarray(
        w.reshape(nh, HEAD_DIM, -1)[:, _ROPE_PERM, :].reshape(w.shape)
    )


def _bf16(a):
    return np.ascontiguousarray(a.astype(ml_dtypes.bfloat16))


def _pmajor(a):
    """[T*128, F...] -> [128, T, F...] partition-major layout."""
    t = a.shape[0] // 128
    return np.ascontiguousarray(
        a.reshape(t, 128, *a.shape[1:]).swapaxes(0, 1)
    )


def make_core_inputs(x, freq_cis, wq, wk, wv, wo):
    """Build the 8 per-core input maps."""
    x = np.asarray(x, np.float32)
    wq = np.asarray(wq, np.float32)
    wk = np.asarray(wk, np.float32)
    wv = np.asarray(wv, np.float32)
    wo = np.asarray(wo, np.float32)
    t1, t2 = _prep_tables(freq_cis)
    masks = _prep_masks()
    ident = _bf16(np.eye(128, dtype=np.float32))
    IQ = HQ * HEAD_DIM

    in_maps = []
    for core in range(8):
        b, g = divmod(core, N_GROUPS)
        wq_g = _perm_head_rows(wq[g * IQ:(g + 1) * IQ])
        wk_g = _perm_head_rows(wk[g * HEAD_DIM:(g + 1) * HEAD_DIM])
        wv_g = wv[g * HEAD_DIM:(g + 1) * HEAD_DIM]
        wqT = _pmajor(_bf16(wq_g.T))
        wkvT = _pmajor(_bf16(np.concatenate([wk_g.T, wv_g.T], axis=1)))
        woT = _pmajor(_bf16(wo[:, g * IQ:(g + 1) * IQ].T))
        in_maps.append({
            "xT": _pmajor(_bf16(x[b].T)),
            "wqT": wqT,
            "wkvT": wkvT,
            "woT": woT,
            "t1": _pmajor(t1),
            "t2": _pmajor(t2),
            "masks": np.ascontiguousarray(masks.swapaxes(0, 1)),
            "ident": ident,
            "ones_col": _bf16(np.ones((128, 1), np.float32)),
            "ones_row": np.ones((1, 128), np.float32),
        })
    return in_maps


_CACHED_NC = None


def _get_nc():
    global _CACHED_NC
    if _CACHED_NC is None:
        from concourse import bacc

        nc = bacc.Bacc("TRN2", target_bir_lowering=False, debug=False)
        build_attention_core(nc)
        nc.compile()
        _CACHED_NC = nc
    return _CACHED_NC


def kernel(x, freq_cis, wq, wk, wv, wo):
    from concourse.bass_utils import run_bass_kernel_spmd

    nc = _get_nc()
    in_maps = make_core_inputs(x, freq_cis, wq, wk, wv, wo)
    res = run_bass_kernel_spmd(nc, in_maps, list(range(8)))
    out = np.zeros((B, S, DIM), dtype=np.float32)
    for core in range(8):
        b = core // N_GROUPS
        out[b] += res.results[core]["out_partial"]
    return out



# revision 28
# speedup vs baseline: 1.5163x; 1.5163x over previous
"""Trainium2 Bass kernel for GQA attention forward (B=2, S=2048, D=2048,
16 q-heads / 4 kv-heads, head_dim=128, RoPE, causal).

Sharding: 8 cores = 2 (batch) x 4 (kv-head groups).  Each core computes its
batch's attention for one kv-head group (4 q-heads + 1 kv head) and a
row-parallel partial of the output projection; the host sums the 4 bf16
partials per batch.

Design (all phases software-pipelined over four 512-row s-chunks):
  * q/k projections are emitted directly in [e, s] (transposed) form
    (lhsT = weight tile, rhs = xT tile) -- no PE transposes anywhere.
  * RoPE runs on DVE in the transposed layout via partition-half swaps.
  * Scores keep keys in partitions / queries free, so exp output feeds the
    PV matmul directly.  Only the 128-wide sub-diagonal block is masked
    (columns are realigned so it is always the first written block).
  * The softmax denominator is a bf16 pair tree on DVE over the exp tiles,
    then a 128-partition sum + reciprocal broadcast on the idle Pool
    engine (partition_all_reduce) -- the tensor engine never touches it.
  * Emission order is A(c) | B(*, c) with the previous chunk's output
    projection C(c-1) interleaved after each head of B(c): C's matmuls
    fill the PE gaps where B is exp-throughput-bound, and its PSUM->SBUF
    copies ride whichever of ACT/DVE has slack in that window.
  * x / weights / RoPE tables stream per-chunk (head-major for wq) so the
    first projection chain starts ~2us in; outputs store as bf16 rows.
PSUM budget is exactly 8 banks: q/out-proj share 2, k 1, scores 3, v/pv 2.
"""

import sys

if "/opt/trn_rl_repo" not in sys.path:
    sys.path.insert(0, "/opt/trn_rl_repo")

import numpy as np
import ml_dtypes

import concourse.bass as bass
import concourse.bass_isa as bass_isa
import concourse.tile as tile
from concourse import mybir

F32 = mybir.dt.float32
F32R = mybir.dt.float32r
BF16 = mybir.dt.bfloat16

# Full-problem constants (per reference).
B, S, DIM = 2, 2048, 2048
N_HEADS, N_KV_HEADS, HEAD_DIM = 16, 4, 128
N_GROUPS = N_KV_HEADS          # tensor-parallel groups
HQ = N_HEADS // N_KV_HEADS     # q heads per group
NEG = -1e30


def build_attention_core(nc, S=S, D=DIM, HQ=HQ, HD=HEAD_DIM, CHUNK=512):
    n_st = S // 128        # s tiles
    n_dt = D // 128        # d tiles
    n_ch = S // CHUNK      # s chunks
    kpc = CHUNK // 128     # k-tiles per chunk
    n_dc = D // CHUNK      # d chunks (phase C)
    spc = CHUNK // 128     # s-tiles per chunk
    IQ = HQ * HD

    x_d = nc.dram_tensor("xT", [128, n_dt, S], BF16, kind="ExternalInput")
    wqT_d = nc.dram_tensor("wqT", [128, HQ, n_dt, HD], BF16,
                           kind="ExternalInput")
    wkvT_d = nc.dram_tensor("wkvT", [128, n_dt, 2 * HD], BF16, kind="ExternalInput")
    woT_d = nc.dram_tensor("woT", [128, IQ // 128, D], BF16, kind="ExternalInput")
    t1_d = nc.dram_tensor("t1", [128, S], BF16, kind="ExternalInput")
    t2_d = nc.dram_tensor("t2", [128, S], BF16, kind="ExternalInput")
    masks_d = nc.dram_tensor("masks", [128, 128], F32, kind="ExternalInput")
    out_d = nc.dram_tensor("out_partial", [S, D], BF16, kind="ExternalOutput")

    scale = float(HD) ** -0.5

    with tile.TileContext(nc) as tc:
        with (
            tc.tile_pool(name="persist", bufs=1) as persist,
            tc.tile_pool(name="xin", bufs=1) as xin_pool,
            tc.tile_pool(name="rope", bufs=4) as rope_pool,
            tc.tile_pool(name="expt", bufs=8) as expt_pool,
            tc.tile_pool(name="acc", bufs=3) as acc_pool,
            tc.tile_pool(name="pairs", bufs=4) as pair_pool,
            tc.tile_pool(name="recip", bufs=3) as rec_pool,
            tc.tile_pool(name="outsb", bufs=4) as outsb_pool,
            # PSUM: 8 banks total
            tc.tile_pool(name="ps_a", bufs=2, space="PSUM") as psa_pool,   # 2
            tc.tile_pool(name="ps_k", bufs=1, space="PSUM") as psk_pool,   # 1
            tc.tile_pool(name="ps_s", bufs=3, space="PSUM") as pss_pool,   # 3
            tc.tile_pool(name="ps_o", bufs=2, space="PSUM") as pso_pool,   # 2
        ):
            # ---------------- weights + constants ---------------------------
            wq_sb = persist.tile([128, HQ, n_dt, HD], BF16)
            wkv_sb = persist.tile([128, n_dt, 2 * HD], BF16)
            t1_sb = persist.tile([128, S], BF16)
            t2_sb = persist.tile([128, S], BF16)
            c0 = slice(0, CHUNK)
            for g in range(n_dt // 4):
                gs = slice(g * 4, (g + 1) * 4)
                nc.scalar.dma_start(out=wkv_sb[:, gs, :], in_=wkvT_d[:, gs, :])
                if g == 0:
                    # chunk-0 RoPE tables early: k-rope needs them ~12us in
                    nc.scalar.dma_start(out=t1_sb[:, c0], in_=t1_d[:, c0])
                    nc.scalar.dma_start(out=t2_sb[:, c0], in_=t2_d[:, c0])
            for h in range(HQ):
                # head-major: q-chain h can start as soon as its slab lands
                nc.scalar.dma_start(out=wq_sb[:, h, :, :], in_=wqT_d[:, h, :, :])
            masks_sb = persist.tile([128, 128], F32)
            nc.scalar.dma_start(out=masks_sb, in_=masks_d[:])
            rest = slice(CHUNK, S)
            nc.scalar.dma_start(out=t1_sb[:, rest], in_=t1_d[:, rest])
            nc.scalar.dma_start(out=t2_sb[:, rest], in_=t2_d[:, rest])
            woT_sb = persist.tile([128, IQ // 128, D], BF16)

            # x streamed per s-chunk on the sync queue
            x_ch = []
            for c in range(n_ch):
                xt = xin_pool.tile([128, n_dt, CHUNK], BF16, tag=f"x{c % 2}",
                                   name=f"x{c}")
                c_sl = slice(c * CHUNK, (c + 1) * CHUNK)
                for g4 in range(n_dt // 4):
                    nc.sync.dma_start(
                        out=xt[:, g4 * 4:(g4 + 1) * 4, :],
                        in_=x_d[:, g4 * 4:(g4 + 1) * 4, c_sl],
                    )
                x_ch.append(xt)
                if c == 1:
                    nc.sync.dma_start(out=woT_sb, in_=woT_d[:])

            # persistent activations
            qT_sb = persist.tile([128, HQ, S], BF16)    # [e, h, s]
            kT_sb = persist.tile([128, S], BF16)        # [e, s]
            v_sb = persist.tile([128, n_st, HD], BF16)  # [s_in_tile, s_tile, e]
            oT_sb = persist.tile([128, HQ, S], BF16)    # [e, h, s]

            # deferred per-(h,c) normalization tail (keeps PE from stalling
            # on the DVE denominator chain)
            norm_pending = [None]

            def emit_norm():
                acc_, ps_o_, h_, c_ = norm_pending[0]
                norm_pending[0] = None
                # softmax denominator: 128-partition sum of the bf16 pair
                # tree, broadcast to all partitions, on the idle Pool engine
                sum_sb = rec_pool.tile([128, CHUNK], F32, tag="sum_sb")
                nc.gpsimd.partition_all_reduce(
                    sum_sb, acc_, channels=128, reduce_op=bass_isa.ReduceOp.add
                )
                rec_sb = rec_pool.tile([128, CHUNK], F32, tag="rec_sb")
                nc.vector.reciprocal_approx_fast(rec_sb, sum_sb)
                nc.vector.tensor_mul(
                    oT_sb[:, h_, c_ * CHUNK:(c_ + 1) * CHUNK], ps_o_, rec_sb
                )

            def emit_out_tile(c, sj, last=False):
                """Phase C for s-tile sj of chunk c: one 128-row output slab.
                Interleaved into B(c+1)'s head loop: its matmuls fill the PE
                gaps where B is exp-throughput-bound, and its PSUM->SBUF
                copies run on DVE (ACT is the B-window pacer).  GPSIMD
                cannot access PSUM, so Pool takes no copies."""
                st = c * spc + sj
                row_sb = outsb_pool.tile([128, D], BF16, tag="out_sb")
                for dc in range(n_dc):
                    if last and dc % 2 == 1:
                        ps_d = pss_pool.tile([128, CHUNK], F32, tag="ps_s")
                    else:
                        ps_d = psa_pool.tile([128, CHUNK], F32, tag="ps_a")
                    for it in range(HQ):
                        nc.tensor.matmul(
                            ps_d,
                            oT_sb[:, it, st * 128:(st + 1) * 128],
                            woT_sb[:, it, dc * CHUNK:(dc + 1) * CHUNK],
                            start=(it == 0), stop=(it == HQ - 1),
                        )
                    dst = row_sb[:, dc * CHUNK:(dc + 1) * CHUNK]
                    # engine choice tracks which engine has slack in the
                    # B window this chunk interleaves with (ACT saturates
                    # as c grows; DVE is flatter)
                    if c == 0:
                        use_act = True
                    elif c == 1:
                        use_act = dc % 2 == 0
                    elif c == 2:
                        use_act = False
                    else:
                        use_act = dc % 2 == 0
                    if use_act:
                        nc.scalar.copy(dst, ps_d)
                    else:
                        nc.vector.tensor_copy(dst, ps_d)
                if last and sj == spc - 1:
                    # final tile: split the store so the tail drains as the
                    # copies complete instead of after the whole row
                    for dc in range(n_dc):
                        nc.sync.dma_start(
                            out=out_d[st * 128:(st + 1) * 128,
                                      dc * CHUNK:(dc + 1) * CHUNK],
                            in_=row_sb[:, dc * CHUNK:(dc + 1) * CHUNK],
                        )
                else:
                    nc.sync.dma_start(
                        out=out_d[st * 128:(st + 1) * 128, :], in_=row_sb
                    )

            def rope(dst, src, c):
                """dst[e, s-chunk] = src*t1 + swap_half(src)*t2 (DVE)."""
                c_sl = slice(c * CHUNK, (c + 1) * CHUNK)
                t1c = t1_sb[:, c_sl]
                t2c = t2_sb[:, c_sl]
                m1 = rope_pool.tile([128, CHUNK], F32, tag="m1")
                nc.vector.tensor_mul(m1, src, t1c)
                m2 = rope_pool.tile([128, CHUNK], F32, tag="m2")
                nc.vector.tensor_mul(m2[0:64, :], src[64:128, :], t2c[0:64, :])
                nc.vector.tensor_mul(m2[64:128, :], src[0:64, :], t2c[64:128, :])
                nc.vector.tensor_add(dst, m1, m2)

            for c in range(n_ch):
                c_sl = slice(c * CHUNK, (c + 1) * CHUNK)
                xt = x_ch[c]

                # ======== A(c): projections + RoPE ========
                ps_k = psk_pool.tile([128, CHUNK], F32, tag="ps_k")
                for dt_ in range(n_dt):
                    nc.tensor.matmul(
                        ps_k, wkv_sb[:, dt_, 0:HD], xt[:, dt_, :],
                        start=(dt_ == 0), stop=(dt_ == n_dt - 1),
                    )
                if norm_pending[0] is not None:
                    emit_norm()
                rope(kT_sb[:, c_sl], ps_k, c)

                for h in range(HQ):
                    ps_qh = psa_pool.tile([128, CHUNK], F32, tag="ps_a")
                    for dt_ in range(n_dt):
                        nc.tensor.matmul(
                            ps_qh, wq_sb[:, h, dt_, :], xt[:, dt_, :],
                            start=(dt_ == 0), stop=(dt_ == n_dt - 1),
                        )
                    rope(qT_sb[:, h, c_sl], ps_qh, c)

                # v: natural [s, e] layout, one chain per s-tile; the
                # four chains share one bank from the ps_o rotation
                ps_vt = pso_pool.tile([128, CHUNK], F32, tag="o")
                for sj in range(spc):
                    st = c * spc + sj
                    sj_sl = slice(sj * 128, (sj + 1) * 128)
                    for dt_ in range(n_dt):
                        nc.tensor.matmul(
                            ps_vt[:, sj_sl], xt[:, dt_, sj_sl],
                            wkv_sb[:, dt_, HD:2 * HD],
                            start=(dt_ == 0), stop=(dt_ == n_dt - 1),
                        )
                    nc.scalar.copy(v_sb[:, st, :], ps_vt[:, sj_sl])

                # ======== B(*, c): attention for q-chunk c ========
                for h in range(HQ):
                    ps_o = pso_pool.tile([128, CHUNK], F32, tag="o")
                    n_kj = (c + 1) * kpc
                    acc = acc_pool.tile([128, CHUNK], BF16, tag="acc")
                    pend_pv = []
                    stash_exp = [None]
                    stash_pair = [None]
                    n_acc = [0]

                    def flush_pv():
                        pe, pj, poff = pend_pv.pop(0)
                        nc.tensor.matmul(
                            ps_o[:, poff:], v_sb[:, pj, :], pe,
                            start=(pj == 0), stop=(pj == n_kj - 1),
                        )

                    for kj in range(n_kj):
                        off = max(0, (kj - c * kpc)) * 128
                        w = CHUNK - off
                        ps_s = pss_pool.tile([128, CHUNK], F32, tag="ps_s")
                        nc.tensor.matmul(
                            ps_s[:, 0:w],
                            kT_sb[:, kj * 128:(kj + 1) * 128],
                            qT_sb[:, h, c * CHUNK + off:(c + 1) * CHUNK],
                            start=True, stop=True,
                        )
                        if kj == 1 and norm_pending[0] is not None:
                            emit_norm()
                        if kj >= c * kpc:
                            # causal mask: ps_s column i holds q-position
                            # off+i, so the partial 128-wide diagonal block
                            # is always the first 128 written columns
                            nc.vector.tensor_add(
                                ps_s[:, 0:128], ps_s[:, 0:128], masks_sb,
                            )
                        expT = expt_pool.tile([128, CHUNK], BF16, tag="expT")
                        if off > 0:
                            # exp output is realigned to q-in-chunk columns;
                            # zero the fully-masked leading columns so the
                            # denominator tree can run full-width
                            nc.gpsimd.memset(expT[:, 0:off], 0.0)
                        nc.scalar.activation(
                            expT[:, off:], ps_s[:, 0:w],
                            mybir.ActivationFunctionType.Exp,
                            scale=scale,
                        )
                        pend_pv.append((expT[:, off:], kj, off))
                        if len(pend_pv) > 3:
                            flush_pv()
                        # denominator: bf16 pair tree on DVE (full width --
                        # masked regions of expT are zeroed above)
                        if kj % 2 == 0:
                            stash_exp[0] = expT
                        else:
                            pr = pair_pool.tile([128, CHUNK], BF16, tag="pair")
                            nc.vector.tensor_add(pr, stash_exp[0], expT)
                            stash_exp[0] = None
                            if n_acc[0] == 0 and stash_pair[0] is None:
                                stash_pair[0] = pr
                            elif n_acc[0] == 0:
                                nc.vector.tensor_add(acc, stash_pair[0], pr)
                                stash_pair[0] = None
                                n_acc[0] = 1
                            else:
                                nc.vector.tensor_add(acc, acc, pr)
                                n_acc[0] += 1
                    while pend_pv:
                        flush_pv()
                    # n_kj is always >= 4 so at least two pairs were formed
                    # and acc is initialized by the second pair.
                    assert n_acc[0] >= 1
                    norm_pending[0] = (acc, ps_o, h, c)
                    if c > 0:
                        emit_out_tile(c - 1, h)

            emit_norm()
            for sj in range(spc):
                emit_out_tile(n_ch - 1, sj, last=True)

    return nc


# ---------------------------------------------------------------------------
# Host-side prep


_ROPE_PERM = np.concatenate([np.arange(0, HEAD_DIM, 2), np.arange(1, HEAD_DIM, 2)])


def _prep_tables(freq_cis):
    """RoPE tables in [e, s] permuted-half layout.

    rot[0:64]   = q[0:64]*cos   + q[64:128]*(-sin)
    rot[64:128] = q[64:128]*cos + q[0:64]*sin
    """
    fc = np.asarray(freq_cis, dtype=np.float32)
    A = fc[:, :, 0, 0]    # cos  [S, 64]
    Bm = fc[:, :, 0, 1]   # -sin
    C = fc[:, :, 1, 0]    # sin
    Dm = fc[:, :, 1, 1]   # cos
    t1 = np.concatenate([A, Dm], axis=1).T    # [128, S]
    t2 = np.concatenate([Bm, C], axis=1).T
    return (_bf16(t1), _bf16(t2))


def _prep_masks():
    q = np.arange(128)[None, :]
    p = np.arange(128)[:, None]
    return np.where(q >= p, np.float32(0.0), np.float32(NEG))


def _perm_head_rows(w):
    """Permute rows within each 128-row head block: evens first, odds second."""
    nh = w.shape[0] // HEAD_DIM
    return np.ascontiguousarray(
        w.reshape(nh, HEAD_DIM, -1)[:, _ROPE_PERM, :].reshape(w.shape)
    )


def _bf16(a):
    return np.ascontiguousarray(a.astype(ml_dtypes.bfloat16))


def _pmajor(a):
    """[T*128, F...] -> [128, T, F...] partition-major layout."""
    t = a.shape[0] // 128
    return np.ascontiguousarray(
        a.reshape(t, 128, *a.shape[1:]).swapaxes(0, 1)
    )


def make_core_inputs(x, freq_cis, wq, wk, wv, wo):
    """Build the 8 per-core input maps."""
    x = np.asarray(x, np.float32)
    wq = np.asarray(wq, np.float32)
    wk = np.asarray(wk, np.float32)
    wv = np.asarray(wv, np.float32)
    wo = np.asarray(wo, np.float32)
    t1, t2 = _prep_tables(freq_cis)
    masks = _prep_masks()
    IQ = HQ * HEAD_DIM

    in_maps = []
    for core in range(8):
        b, g = divmod(core, N_GROUPS)
        wq_g = _perm_head_rows(wq[g * IQ:(g + 1) * IQ])
        wk_g = _perm_head_rows(wk[g * HEAD_DIM:(g + 1) * HEAD_DIM])
        wv_g = wv[g * HEAD_DIM:(g + 1) * HEAD_DIM]
        # [D, IQ] -> [128, dt, IQ] -> [128, HQ, dt, HD] head-major
        wqT = _pmajor(_bf16(wq_g.T)).reshape(128, 16, HQ, HEAD_DIM)
        wqT = np.ascontiguousarray(wqT.swapaxes(1, 2))
        wkvT = _pmajor(_bf16(np.concatenate([wk_g.T, wv_g.T], axis=1)))
        woT = _pmajor(_bf16(wo[:, g * IQ:(g + 1) * IQ].T))
        in_maps.append({
            "xT": _pmajor(_bf16(x[b].T)),
            "wqT": wqT,
            "wkvT": wkvT,
            "woT": woT,
            "t1": t1,
            "t2": t2,
            "masks": np.ascontiguousarray(masks),
        })
    return in_maps


_CACHED_NC = None


def _get_nc():
    global _CACHED_NC
    if _CACHED_NC is None:
        from concourse import bacc

        nc = bacc.Bacc("TRN2", target_bir_lowering=False, debug=False)
        build_attention_core(nc)
        nc.compile()
        _CACHED_NC = nc
    return _CACHED_NC


def kernel(x, freq_cis, wq, wk, wv, wo):
    from concourse.bass_utils import run_bass_kernel_spmd

    nc = _get_nc()
    in_maps = make_core_inputs(x, freq_cis, wq, wk, wv, wo)
    res = run_bass_kernel_spmd(nc, in_maps, list(range(8)))
    out = np.zeros((B, S, DIM), dtype=np.float32)
    for core in range(8):
        b = core // N_GROUPS
        out[b] += res.results[core]["out_partial"].astype(np.float32)
    return out


# revision 36
# speedup vs baseline: 1.5164x; 1.0001x over previous
"""Trainium2 Bass kernel for GQA attention forward (B=2, S=2048, D=2048,
16 q-heads / 4 kv-heads, head_dim=128, RoPE, causal).

Sharding: 8 cores = 2 (batch) x 4 (kv-head groups).  Each core computes its
batch's attention for one kv-head group (4 q-heads + 1 kv head) and a
row-parallel partial of the output projection; the host sums the 4 bf16
partials per batch.

Design (all phases software-pipelined over four 512-row s-chunks):
  * q/k projections are emitted directly in [e, s] (transposed) form
    (lhsT = weight tile, rhs = xT tile) -- no PE transposes anywhere.
  * RoPE runs on DVE in the transposed layout via partition-half swaps.
  * Scores keep keys in partitions / queries free, so exp output feeds the
    PV matmul directly.  Only the 128-wide sub-diagonal block is masked
    (columns are realigned so it is always the first written block).
  * The softmax denominator is a bf16 pair tree on DVE over the exp tiles,
    then a 128-partition sum + reciprocal broadcast on the idle Pool
    engine (partition_all_reduce) -- the tensor engine never touches it.
  * Emission order is A(c) | B(*, c) with the previous chunk's output
    projection C(c-1) interleaved after each head of B(c): C's matmuls
    fill the PE gaps where B is exp-throughput-bound, and its PSUM->SBUF
    copies ride whichever of ACT/DVE has slack in that window.
  * x / weights / RoPE tables stream per-chunk (head-major for wq) so the
    first projection chain starts ~2us in; outputs store as bf16 rows.
PSUM budget is exactly 8 banks: k/q/out-proj share 2, scores 4, v/pv 2.
"""

import sys

if "/opt/trn_rl_repo" not in sys.path:
    sys.path.insert(0, "/opt/trn_rl_repo")

import numpy as np
import ml_dtypes

import concourse.bass as bass
import concourse.bass_isa as bass_isa
import concourse.tile as tile
from concourse import mybir

F32 = mybir.dt.float32
F32R = mybir.dt.float32r
BF16 = mybir.dt.bfloat16

# Full-problem constants (per reference).
B, S, DIM = 2, 2048, 2048
N_HEADS, N_KV_HEADS, HEAD_DIM = 16, 4, 128
N_GROUPS = N_KV_HEADS          # tensor-parallel groups
HQ = N_HEADS // N_KV_HEADS     # q heads per group
NEG = -1e30


def build_attention_core(nc, S=S, D=DIM, HQ=HQ, HD=HEAD_DIM, CHUNK=512):
    n_st = S // 128        # s tiles
    n_dt = D // 128        # d tiles
    n_ch = S // CHUNK      # s chunks
    kpc = CHUNK // 128     # k-tiles per chunk
    n_dc = D // CHUNK      # d chunks (phase C)
    spc = CHUNK // 128     # s-tiles per chunk
    IQ = HQ * HD

    x_d = nc.dram_tensor("xT", [128, n_dt, S], BF16, kind="ExternalInput")
    wqT_d = nc.dram_tensor("wqT", [128, HQ, n_dt, HD], BF16,
                           kind="ExternalInput")
    wkvT_d = nc.dram_tensor("wkvT", [128, n_dt, 2 * HD], BF16, kind="ExternalInput")
    woT_d = nc.dram_tensor("woT", [128, IQ // 128, D], BF16, kind="ExternalInput")
    t1_d = nc.dram_tensor("t1", [128, S], BF16, kind="ExternalInput")
    t2_d = nc.dram_tensor("t2", [128, S], BF16, kind="ExternalInput")
    masks_d = nc.dram_tensor("masks", [128, 128], F32, kind="ExternalInput")
    out_d = nc.dram_tensor("out_partial", [S, D], BF16, kind="ExternalOutput")

    scale = float(HD) ** -0.5

    with tile.TileContext(nc) as tc:
        with (
            tc.tile_pool(name="persist", bufs=1) as persist,
            tc.tile_pool(name="xin", bufs=1) as xin_pool,
            tc.tile_pool(name="rope", bufs=4) as rope_pool,
            tc.tile_pool(name="expt", bufs=8) as expt_pool,
            tc.tile_pool(name="acc", bufs=3) as acc_pool,
            tc.tile_pool(name="pairs", bufs=4) as pair_pool,
            tc.tile_pool(name="recip", bufs=3) as rec_pool,
            tc.tile_pool(name="outsb", bufs=4) as outsb_pool,
            # PSUM: 8 banks total
            tc.tile_pool(name="ps_a", bufs=2, space="PSUM") as psa_pool,   # 2
            tc.tile_pool(name="ps_s", bufs=4, space="PSUM") as pss_pool,   # 4
            tc.tile_pool(name="ps_o", bufs=2, space="PSUM") as pso_pool,   # 2
        ):
            # ---------------- weights + constants ---------------------------
            wq_sb = persist.tile([128, HQ, n_dt, HD], BF16)
            wkv_sb = persist.tile([128, n_dt, 2 * HD], BF16)
            t1_sb = persist.tile([128, S], BF16)
            t2_sb = persist.tile([128, S], BF16)
            c0 = slice(0, CHUNK)
            for g in range(n_dt // 4):
                gs = slice(g * 4, (g + 1) * 4)
                nc.scalar.dma_start(out=wkv_sb[:, gs, :], in_=wkvT_d[:, gs, :])
                if g == 0:
                    # chunk-0 RoPE tables early: k-rope needs them ~12us in
                    nc.scalar.dma_start(out=t1_sb[:, c0], in_=t1_d[:, c0])
                    nc.scalar.dma_start(out=t2_sb[:, c0], in_=t2_d[:, c0])
            for h in range(HQ):
                # head-major: q-chain h can start as soon as its slab lands
                nc.scalar.dma_start(out=wq_sb[:, h, :, :], in_=wqT_d[:, h, :, :])
            masks_sb = persist.tile([128, 128], F32)
            nc.scalar.dma_start(out=masks_sb, in_=masks_d[:])
            rest = slice(CHUNK, S)
            nc.scalar.dma_start(out=t1_sb[:, rest], in_=t1_d[:, rest])
            nc.scalar.dma_start(out=t2_sb[:, rest], in_=t2_d[:, rest])
            woT_sb = persist.tile([128, IQ // 128, D], BF16)

            # x streamed per s-chunk on the sync queue
            x_ch = []
            for c in range(n_ch):
                xt = xin_pool.tile([128, n_dt, CHUNK], BF16, tag=f"x{c % 2}",
                                   name=f"x{c}")
                c_sl = slice(c * CHUNK, (c + 1) * CHUNK)
                ng = 2 if c == 0 else 4
                for g4 in range(n_dt // ng):
                    nc.sync.dma_start(
                        out=xt[:, g4 * ng:(g4 + 1) * ng, :],
                        in_=x_d[:, g4 * ng:(g4 + 1) * ng, c_sl],
                    )
                x_ch.append(xt)
                if c == 1:
                    nc.sync.dma_start(out=woT_sb, in_=woT_d[:])

            # persistent activations
            qT_sb = persist.tile([128, HQ, S], BF16)    # [e, h, s]
            kT_sb = persist.tile([128, S], BF16)        # [e, s]
            v_sb = persist.tile([128, n_st, HD], BF16)  # [s_in_tile, s_tile, e]
            oT_sb = persist.tile([128, HQ, S], BF16)    # [e, h, s]

            # deferred per-(h,c) normalization tail (keeps PE from stalling
            # on the DVE denominator chain)
            norm_pending = [None]

            def emit_norm():
                acc_, ps_o_, h_, c_ = norm_pending[0]
                norm_pending[0] = None
                # softmax denominator: 128-partition sum of the bf16 pair
                # tree, broadcast to all partitions, on the idle Pool engine
                sum_sb = rec_pool.tile([128, CHUNK], F32, tag="sum_sb")
                nc.gpsimd.partition_all_reduce(
                    sum_sb, acc_, channels=128, reduce_op=bass_isa.ReduceOp.add
                )
                rec_sb = rec_pool.tile([128, CHUNK], F32, tag="rec_sb")
                nc.vector.reciprocal_approx_fast(rec_sb, sum_sb)
                nc.vector.tensor_mul(
                    oT_sb[:, h_, c_ * CHUNK:(c_ + 1) * CHUNK], ps_o_, rec_sb
                )

            def emit_out_tile(c, sj, last=False):
                """Phase C for s-tile sj of chunk c: one 128-row output slab.
                Interleaved into B(c+1)'s head loop: its matmuls fill the PE
                gaps where B is exp-throughput-bound, and its PSUM->SBUF
                copies run on DVE (ACT is the B-window pacer).  GPSIMD
                cannot access PSUM, so Pool takes no copies."""
                st = c * spc + sj
                row_sb = outsb_pool.tile([128, D], BF16, tag="out_sb")
                for dc in range(n_dc):
                    if last and dc % 2 == 1:
                        ps_d = pss_pool.tile([128, CHUNK], F32, tag="ps_s")
                    else:
                        ps_d = psa_pool.tile([128, CHUNK], F32, tag="ps_a")
                    for it in range(HQ):
                        nc.tensor.matmul(
                            ps_d,
                            oT_sb[:, it, st * 128:(st + 1) * 128],
                            woT_sb[:, it, dc * CHUNK:(dc + 1) * CHUNK],
                            start=(it == 0), stop=(it == HQ - 1),
                        )
                    dst = row_sb[:, dc * CHUNK:(dc + 1) * CHUNK]
                    # engine choice tracks which engine has slack in the
                    # B window this chunk interleaves with (ACT saturates
                    # as c grows; DVE is flatter)
                    if c == 0:
                        use_act = True
                    elif c == 1:
                        use_act = dc % 2 == 0
                    elif c == 2:
                        use_act = False
                    else:
                        use_act = dc % 2 == 0
                    if use_act:
                        nc.scalar.copy(dst, ps_d)
                    else:
                        nc.vector.tensor_copy(dst, ps_d)
                if last and sj == spc - 1:
                    # final tile: split the store so the tail drains as the
                    # copies complete instead of after the whole row
                    for dc in range(n_dc):
                        nc.sync.dma_start(
                            out=out_d[st * 128:(st + 1) * 128,
                                      dc * CHUNK:(dc + 1) * CHUNK],
                            in_=row_sb[:, dc * CHUNK:(dc + 1) * CHUNK],
                        )
                else:
                    nc.sync.dma_start(
                        out=out_d[st * 128:(st + 1) * 128, :], in_=row_sb
                    )

            def rope(dst, src, c):
                """dst[e, s-chunk] = src*t1 + swap_half(src)*t2 (DVE)."""
                c_sl = slice(c * CHUNK, (c + 1) * CHUNK)
                t1c = t1_sb[:, c_sl]
                t2c = t2_sb[:, c_sl]
                m1 = rope_pool.tile([128, CHUNK], F32, tag="m1")
                nc.vector.tensor_mul(m1, src, t1c)
                m2 = rope_pool.tile([128, CHUNK], F32, tag="m2")
                nc.vector.tensor_mul(m2[0:64, :], src[64:128, :], t2c[0:64, :])
                nc.vector.tensor_mul(m2[64:128, :], src[0:64, :], t2c[64:128, :])
                nc.vector.tensor_add(dst, m1, m2)

            for c in range(n_ch):
                c_sl = slice(c * CHUNK, (c + 1) * CHUNK)
                xt = x_ch[c]

                # ======== A(c): projections + RoPE ========
                ps_k = psa_pool.tile([128, CHUNK], F32, tag="ps_a")
                for dt_ in range(n_dt):
                    nc.tensor.matmul(
                        ps_k, wkv_sb[:, dt_, 0:HD], xt[:, dt_, :],
                        start=(dt_ == 0), stop=(dt_ == n_dt - 1),
                    )
                if norm_pending[0] is not None:
                    emit_norm()
                rope(kT_sb[:, c_sl], ps_k, c)

                for h in range(HQ):
                    ps_qh = psa_pool.tile([128, CHUNK], F32, tag="ps_a")
                    for dt_ in range(n_dt):
                        nc.tensor.matmul(
                            ps_qh, wq_sb[:, h, dt_, :], xt[:, dt_, :],
                            start=(dt_ == 0), stop=(dt_ == n_dt - 1),
                        )
                    rope(qT_sb[:, h, c_sl], ps_qh, c)

                # v: natural [s, e] layout, one chain per s-tile; the
                # four chains share one bank from the ps_o rotation
                ps_vt = pso_pool.tile([128, CHUNK], F32, tag="o")
                for sj in range(spc):
                    st = c * spc + sj
                    sj_sl = slice(sj * 128, (sj + 1) * 128)
                    for dt_ in range(n_dt):
                        nc.tensor.matmul(
                            ps_vt[:, sj_sl], xt[:, dt_, sj_sl],
                            wkv_sb[:, dt_, HD:2 * HD],
                            start=(dt_ == 0), stop=(dt_ == n_dt - 1),
                        )
                    nc.scalar.copy(v_sb[:, st, :], ps_vt[:, sj_sl])

                # ======== B(*, c): attention for q-chunk c ========
                for h in range(HQ):
                    ps_o = pso_pool.tile([128, CHUNK], F32, tag="o")
                    n_kj = (c + 1) * kpc
                    acc = acc_pool.tile([128, CHUNK], BF16, tag="acc")
                    pend_pv = []
                    stash_exp = [None]
                    stash_pair = [None]
                    n_acc = [0]

                    def flush_pv():
                        pe, pj, poff = pend_pv.pop(0)
                        nc.tensor.matmul(
                            ps_o[:, poff:], v_sb[:, pj, :], pe,
                            start=(pj == 0), stop=(pj == n_kj - 1),
                        )

                    for kj in range(n_kj):
                        off = max(0, (kj - c * kpc)) * 128
                        w = CHUNK - off
                        ps_s = pss_pool.tile([128, CHUNK], F32, tag="ps_s")
                        nc.tensor.matmul(
                            ps_s[:, 0:w],
                            kT_sb[:, kj * 128:(kj + 1) * 128],
                            qT_sb[:, h, c * CHUNK + off:(c + 1) * CHUNK],
                            start=True, stop=True,
                        )
                        if kj == 1 and norm_pending[0] is not None:
                            emit_norm()
                        if kj >= c * kpc:
                            # causal mask: ps_s column i holds q-position
                            # off+i, so the partial 128-wide diagonal block
                            # is always the first 128 written columns
                            nc.vector.tensor_add(
                                ps_s[:, 0:128], ps_s[:, 0:128], masks_sb,
                            )
                        expT = expt_pool.tile([128, CHUNK], BF16, tag="expT")
                        if off > 0:
                            # exp output is realigned to q-in-chunk columns;
                            # zero the fully-masked leading columns so the
                            # denominator tree can run full-width
                            nc.gpsimd.memset(expT[:, 0:off], 0.0)
                        nc.scalar.activation(
                            expT[:, off:], ps_s[:, 0:w],
                            mybir.ActivationFunctionType.Exp,
                            scale=scale,
                        )
                        pend_pv.append((expT[:, off:], kj, off))
                        if len(pend_pv) > 3:
                            flush_pv()
                        # denominator: bf16 pair tree on DVE (full width --
                        # masked regions of expT are zeroed above)
                        if kj % 2 == 0:
                            stash_exp[0] = expT
                        else:
                            pr = pair_pool.tile([128, CHUNK], BF16, tag="pair")
                            nc.vector.tensor_add(pr, stash_exp[0], expT)
                            stash_exp[0] = None
                            if n_acc[0] == 0 and stash_pair[0] is None:
                                stash_pair[0] = pr
                            elif n_acc[0] == 0:
                                nc.vector.tensor_add(acc, stash_pair[0], pr)
                                stash_pair[0] = None
                                n_acc[0] = 1
                            else:
                                nc.vector.tensor_add(acc, acc, pr)
                                n_acc[0] += 1
                    while pend_pv:
                        flush_pv()
                    # n_kj is always >= 4 so at least two pairs were formed
                    # and acc is initialized by the second pair.
                    assert n_acc[0] >= 1
                    norm_pending[0] = (acc, ps_o, h, c)
                    if c > 0:
                        emit_out_tile(c - 1, h)

            emit_norm()
            for sj in range(spc):
                emit_out_tile(n_ch - 1, sj, last=True)

    return nc


# ---------------------------------------------------------------------------
# Host-side prep


_ROPE_PERM = np.concatenate([np.arange(0, HEAD_DIM, 2), np.arange(1, HEAD_DIM, 2)])


def _prep_tables(freq_cis):
    """RoPE tables in [e, s] permuted-half layout.

    rot[0:64]   = q[0:64]*cos   + q[64:128]*(-sin)
    rot[64:128] = q[64:128]*cos + q[0:64]*sin
    """
    fc = np.asarray(freq_cis, dtype=np.float32)
    A = fc[:, :, 0, 0]    # cos  [S, 64]
    Bm = fc[:, :, 0, 1]   # -sin
    C = fc[:, :, 1, 0]    # sin
    Dm = fc[:, :, 1, 1]   # cos
    t1 = np.concatenate([A, Dm], axis=1).T    # [128, S]
    t2 = np.concatenate([Bm, C], axis=1).T
    return (_bf16(t1), _bf16(t2))


def _prep_masks():
    q = np.arange(128)[None, :]
    p = np.arange(128)[:, None]
    return np.where(q >= p, np.float32(0.0), np.float32(NEG))


def _perm_head_rows(w):
    """Permute rows within each 128-row head block: evens first, odds second."""
    nh = w.shape[0] // HEAD_DIM
    return np.ascontiguousarray(
        w.reshape(nh, HEAD_DIM, -1)[:, _ROPE_PERM, :].reshape(w.shape)
    )


def _bf16(a):
    return np.ascontiguousarray(a.astype(ml_dtypes.bfloat16))


def _pmajor(a):
    """[T*128, F...] -> [128, T, F...] partition-major layout."""
    t = a.shape[0] // 128
    return np.ascontiguousarray(
        a.reshape(t, 128, *a.shape[1:]).swapaxes(0, 1)
    )


def make_core_inputs(x, freq_cis, wq, wk, wv, wo):
    """Build the 8 per-core input maps."""
    x = np.asarray(x, np.float32)
    wq = np.asarray(wq, np.float32)
    wk = np.asarray(wk, np.float32)
    wv = np.asarray(wv, np.float32)
    wo = np.asarray(wo, np.float32)
    t1, t2 = _prep_tables(freq_cis)
    masks = _prep_masks()
    IQ = HQ * HEAD_DIM

    in_maps = []
    for core in range(8):
        b, g = divmod(core, N_GROUPS)
        wq_g = _perm_head_rows(wq[g * IQ:(g + 1) * IQ])
        wk_g = _perm_head_rows(wk[g * HEAD_DIM:(g + 1) * HEAD_DIM])
        wv_g = wv[g * HEAD_DIM:(g + 1) * HEAD_DIM]
        # [D, IQ] -> [128, dt, IQ] -> [128, HQ, dt, HD] head-major
        wqT = _pmajor(_bf16(wq_g.T)).reshape(128, 16, HQ, HEAD_DIM)
        wqT = np.ascontiguousarray(wqT.swapaxes(1, 2))
        wkvT = _pmajor(_bf16(np.concatenate([wk_g.T, wv_g.T], axis=1)))
        woT = _pmajor(_bf16(wo[:, g * IQ:(g + 1) * IQ].T))
        in_maps.append({
            "xT": _pmajor(_bf16(x[b].T)),
            "wqT": wqT,
            "wkvT": wkvT,
            "woT": woT,
            "t1": t1,
            "t2": t2,
            "masks": np.ascontiguousarray(masks),
        })
    return in_maps


_CACHED_NC = None


def _get_nc():
    global _CACHED_NC
    if _CACHED_NC is None:
        from concourse import bacc

        nc = bacc.Bacc("TRN2", target_bir_lowering=False, debug=False)
        build_attention_core(nc)
        nc.compile()
        _CACHED_NC = nc
    return _CACHED_NC


def kernel(x, freq_cis, wq, wk, wv, wo):
    from concourse.bass_utils import run_bass_kernel_spmd

    nc = _get_nc()
    in_maps = make_core_inputs(x, freq_cis, wq, wk, wv, wo)
    res = run_bass_kernel_spmd(nc, in_maps, list(range(8)))
    out = np.zeros((B, S, DIM), dtype=np.float32)
    for core in range(8):
        b = core // N_GROUPS
        out[b] += res.results[core]["out_partial"].astype(np.float32)
    return out


# revision 41
# speedup vs baseline: 1.5175x; 1.0007x over previous
"""Trainium2 Bass kernel for GQA attention forward (B=2, S=2048, D=2048,
16 q-heads / 4 kv-heads, head_dim=128, RoPE, causal).

Sharding: 8 cores = 2 (batch) x 4 (kv-head groups).  Each core computes its
batch's attention for one kv-head group (4 q-heads + 1 kv head) and a
row-parallel partial of the output projection; the host sums the 4 bf16
partials per batch.

Design (all phases software-pipelined over four 512-row s-chunks):
  * q/k projections are emitted directly in [e, s] (transposed) form
    (lhsT = weight tile, rhs = xT tile) -- no PE transposes anywhere.
  * RoPE runs on DVE in the transposed layout via partition-half swaps.
  * Scores keep keys in partitions / queries free, so exp output feeds the
    PV matmul directly.  Only the 128-wide sub-diagonal block is masked
    (columns are realigned so it is always the first written block).
  * The softmax denominator is a bf16 pair tree on DVE over the exp tiles,
    then a 128-partition sum + reciprocal broadcast on the idle Pool
    engine (partition_all_reduce) -- the tensor engine never touches it.
  * Emission order is A(c) | B(*, c) with the previous chunk's output
    projection C(c-1) interleaved after each head of B(c): C's matmuls
    fill the PE gaps where B is exp-throughput-bound, and its PSUM->SBUF
    copies ride whichever of ACT/DVE has slack in that window.
  * x / weights / RoPE tables stream per-chunk (head-major for wq) so the
    first projection chain starts ~2us in; outputs store as bf16 rows.
PSUM budget is exactly 8 banks: k/q/out-proj share 2, scores 4, v/pv 2.
"""

import sys

if "/opt/trn_rl_repo" not in sys.path:
    sys.path.insert(0, "/opt/trn_rl_repo")

import numpy as np
import ml_dtypes

import concourse.bass as bass
import concourse.bass_isa as bass_isa
import concourse.tile as tile
from concourse import mybir

F32 = mybir.dt.float32
F32R = mybir.dt.float32r
BF16 = mybir.dt.bfloat16

# Full-problem constants (per reference).
B, S, DIM = 2, 2048, 2048
N_HEADS, N_KV_HEADS, HEAD_DIM = 16, 4, 128
N_GROUPS = N_KV_HEADS          # tensor-parallel groups
HQ = N_HEADS // N_KV_HEADS     # q heads per group
NEG = -1e30


def build_attention_core(nc, S=S, D=DIM, HQ=HQ, HD=HEAD_DIM, CHUNK=512):
    n_st = S // 128        # s tiles
    n_dt = D // 128        # d tiles
    n_ch = S // CHUNK      # s chunks
    kpc = CHUNK // 128     # k-tiles per chunk
    n_dc = D // CHUNK      # d chunks (phase C)
    spc = CHUNK // 128     # s-tiles per chunk
    IQ = HQ * HD

    x_d = nc.dram_tensor("xT", [128, n_dt, S], BF16, kind="ExternalInput")
    wqT_d = nc.dram_tensor("wqT", [128, HQ, n_dt, HD], BF16,
                           kind="ExternalInput")
    wkvT_d = nc.dram_tensor("wkvT", [128, n_dt, 2 * HD], BF16, kind="ExternalInput")
    woT_d = nc.dram_tensor("woT", [128, IQ // 128, D], BF16, kind="ExternalInput")
    t1_d = nc.dram_tensor("t1", [128, S], BF16, kind="ExternalInput")
    t2_d = nc.dram_tensor("t2", [128, S], BF16, kind="ExternalInput")
    masks_d = nc.dram_tensor("masks", [128, 128], F32, kind="ExternalInput")
    out_d = nc.dram_tensor("out_partial", [S, D], BF16, kind="ExternalOutput")

    scale = float(HD) ** -0.5

    with tile.TileContext(nc) as tc:
        with (
            tc.tile_pool(name="persist", bufs=1) as persist,
            tc.tile_pool(name="xin", bufs=1) as xin_pool,
            tc.tile_pool(name="rope", bufs=4) as rope_pool,
            tc.tile_pool(name="expt", bufs=8) as expt_pool,
            tc.tile_pool(name="acc", bufs=3) as acc_pool,
            tc.tile_pool(name="pairs", bufs=4) as pair_pool,
            tc.tile_pool(name="recip", bufs=3) as rec_pool,
            tc.tile_pool(name="outsb", bufs=4) as outsb_pool,
            # PSUM: 8 banks total
            tc.tile_pool(name="ps_a", bufs=2, space="PSUM") as psa_pool,   # 2
            tc.tile_pool(name="ps_s", bufs=4, space="PSUM") as pss_pool,   # 4
            tc.tile_pool(name="ps_o", bufs=2, space="PSUM") as pso_pool,   # 2
        ):
            # ---------------- weights + constants ---------------------------
            wq_sb = persist.tile([128, HQ, n_dt, HD], BF16)
            wkv_sb = persist.tile([128, n_dt, 2 * HD], BF16)
            t1_sb = persist.tile([128, S], BF16)
            t2_sb = persist.tile([128, S], BF16)
            c0 = slice(0, CHUNK)
            for g in range(n_dt // 4):
                gs = slice(g * 4, (g + 1) * 4)
                nc.scalar.dma_start(out=wkv_sb[:, gs, :], in_=wkvT_d[:, gs, :])
                if g == 0:
                    # chunk-0 RoPE tables early: k-rope needs them ~12us in
                    nc.scalar.dma_start(out=t1_sb[:, c0], in_=t1_d[:, c0])
                    nc.scalar.dma_start(out=t2_sb[:, c0], in_=t2_d[:, c0])
            for h in range(HQ):
                # head-major: q-chain h can start as soon as its slab lands
                nc.scalar.dma_start(out=wq_sb[:, h, :, :], in_=wqT_d[:, h, :, :])
            masks_sb = persist.tile([128, 128], F32)
            nc.scalar.dma_start(out=masks_sb, in_=masks_d[:])
            rest = slice(CHUNK, S)
            nc.scalar.dma_start(out=t1_sb[:, rest], in_=t1_d[:, rest])
            nc.scalar.dma_start(out=t2_sb[:, rest], in_=t2_d[:, rest])
            woT_sb = persist.tile([128, IQ // 128, D], BF16)

            # x streamed per s-chunk on the sync queue
            x_ch = []
            for c in range(n_ch):
                xt = xin_pool.tile([128, n_dt, CHUNK], BF16, tag=f"x{c % 2}",
                                   name=f"x{c}")
                c_sl = slice(c * CHUNK, (c + 1) * CHUNK)
                ng = 2 if c == 0 else 4
                for g4 in range(n_dt // ng):
                    nc.sync.dma_start(
                        out=xt[:, g4 * ng:(g4 + 1) * ng, :],
                        in_=x_d[:, g4 * ng:(g4 + 1) * ng, c_sl],
                    )
                x_ch.append(xt)
                if c == 1:
                    nc.sync.dma_start(out=woT_sb, in_=woT_d[:])

            # persistent activations
            qT_sb = persist.tile([128, HQ, S], BF16)    # [e, h, s]
            kT_sb = persist.tile([128, S], BF16)        # [e, s]
            v_sb = persist.tile([128, n_st, HD], BF16)  # [s_in_tile, s_tile, e]
            oT_sb = persist.tile([128, HQ, S], BF16)    # [e, h, s]

            # deferred per-(h,c) normalization tail (keeps PE from stalling
            # on the DVE denominator chain)
            norm_pending = [None]

            def emit_norm():
                acc_, ps_o_, h_, c_ = norm_pending[0]
                norm_pending[0] = None
                # softmax denominator: 128-partition sum of the bf16 pair
                # tree, broadcast to all partitions, on the idle Pool engine
                sum_sb = rec_pool.tile([128, CHUNK], F32, tag="sum_sb")
                nc.gpsimd.partition_all_reduce(
                    sum_sb, acc_, channels=128, reduce_op=bass_isa.ReduceOp.add
                )
                rec_sb = rec_pool.tile([128, CHUNK], F32, tag="rec_sb")
                nc.vector.reciprocal_approx_fast(rec_sb, sum_sb)
                nc.vector.tensor_mul(
                    oT_sb[:, h_, c_ * CHUNK:(c_ + 1) * CHUNK], ps_o_, rec_sb
                )

            def emit_out_tile(c, sj, last=False):
                """Phase C for s-tile sj of chunk c: one 128-row output slab.
                Interleaved into B(c+1)'s head loop: its matmuls fill the PE
                gaps where B is exp-throughput-bound, and its PSUM->SBUF
                copies run on DVE (ACT is the B-window pacer).  GPSIMD
                cannot access PSUM, so Pool takes no copies."""
                st = c * spc + sj
                row_sb = outsb_pool.tile([128, D], BF16, tag="out_sb")
                for dc in range(n_dc):
                    if last and dc % 2 == 1:
                        ps_d = pss_pool.tile([128, CHUNK], F32, tag="ps_s")
                    else:
                        ps_d = psa_pool.tile([128, CHUNK], F32, tag="ps_a")
                    for it in range(HQ):
                        nc.tensor.matmul(
                            ps_d,
                            oT_sb[:, it, st * 128:(st + 1) * 128],
                            woT_sb[:, it, dc * CHUNK:(dc + 1) * CHUNK],
                            start=(it == 0), stop=(it == HQ - 1),
                        )
                    dst = row_sb[:, dc * CHUNK:(dc + 1) * CHUNK]
                    # engine choice tracks which engine has slack in the
                    # B window this chunk interleaves with (ACT saturates
                    # as c grows; DVE is flatter)
                    if c == 0:
                        use_act = True
                    elif c == 1:
                        use_act = dc % 2 == 0
                    elif c == 2:
                        use_act = False
                    else:
                        use_act = dc % 2 == 0
                    if use_act:
                        nc.scalar.copy(dst, ps_d)
                    else:
                        nc.vector.tensor_copy(dst, ps_d)
                if last and sj == spc - 1:
                    # final tile: split the store so the tail drains as the
                    # copies complete instead of after the whole row
                    for dc in range(n_dc):
                        nc.sync.dma_start(
                            out=out_d[st * 128:(st + 1) * 128,
                                      dc * CHUNK:(dc + 1) * CHUNK],
                            in_=row_sb[:, dc * CHUNK:(dc + 1) * CHUNK],
                        )
                else:
                    nc.sync.dma_start(
                        out=out_d[st * 128:(st + 1) * 128, :], in_=row_sb
                    )

            def rope(dst, src, c):
                """dst[e, s-chunk] = src*t1 + swap_half(src)*t2 (DVE)."""
                c_sl = slice(c * CHUNK, (c + 1) * CHUNK)
                t1c = t1_sb[:, c_sl]
                t2c = t2_sb[:, c_sl]
                m1 = rope_pool.tile([128, CHUNK], F32, tag="m1")
                nc.vector.tensor_mul(m1, src, t1c)
                m2 = rope_pool.tile([128, CHUNK], F32, tag="m2")
                nc.vector.tensor_mul(m2[0:64, :], src[64:128, :], t2c[0:64, :])
                nc.vector.tensor_mul(m2[64:128, :], src[0:64, :], t2c[64:128, :])
                nc.vector.tensor_add(dst, m1, m2)

            for c in range(n_ch):
                c_sl = slice(c * CHUNK, (c + 1) * CHUNK)
                xt = x_ch[c]

                # ======== A(c): projections + RoPE ========
                ps_k = psa_pool.tile([128, CHUNK], F32, tag="ps_a")
                for dt_ in range(n_dt):
                    nc.tensor.matmul(
                        ps_k, wkv_sb[:, dt_, 0:HD], xt[:, dt_, :],
                        start=(dt_ == 0), stop=(dt_ == n_dt - 1),
                    )
                if norm_pending[0] is not None:
                    emit_norm()
                rope(kT_sb[:, c_sl], ps_k, c)

                for h in range(HQ):
                    ps_qh = psa_pool.tile([128, CHUNK], F32, tag="ps_a")
                    for dt_ in range(n_dt):
                        nc.tensor.matmul(
                            ps_qh, wq_sb[:, h, dt_, :], xt[:, dt_, :],
                            start=(dt_ == 0), stop=(dt_ == n_dt - 1),
                        )
                    rope(qT_sb[:, h, c_sl], ps_qh, c)

                # v: natural [s, e] layout, one chain per s-tile; the
                # four chains share one bank from the ps_o rotation
                ps_vt = pso_pool.tile([128, CHUNK], F32, tag="o")
                for sj in range(spc):
                    st = c * spc + sj
                    sj_sl = slice(sj * 128, (sj + 1) * 128)
                    for dt_ in range(n_dt):
                        nc.tensor.matmul(
                            ps_vt[:, sj_sl], xt[:, dt_, sj_sl],
                            wkv_sb[:, dt_, HD:2 * HD],
                            start=(dt_ == 0), stop=(dt_ == n_dt - 1),
                        )
                    nc.scalar.copy(v_sb[:, st, :], ps_vt[:, sj_sl])

                # ======== B(*, c): attention for q-chunk c ========
                for h in range(HQ):
                    ps_o = pso_pool.tile([128, CHUNK], F32, tag="o")
                    n_kj = (c + 1) * kpc
                    acc = acc_pool.tile([128, CHUNK], BF16, tag="acc")
                    pend_pv = []
                    stash_exp = [None]
                    stash_pair = [None]
                    n_acc = [0]

                    def flush_pv():
                        pe, pj, poff = pend_pv.pop(0)
                        nc.tensor.matmul(
                            ps_o[:, poff:], v_sb[:, pj, :], pe,
                            start=(pj == 0), stop=(pj == n_kj - 1),
                        )

                    for kj in range(n_kj):
                        off = max(0, (kj - c * kpc)) * 128
                        w = CHUNK - off
                        ps_s = pss_pool.tile([128, CHUNK], F32, tag="ps_s")
                        nc.tensor.matmul(
                            ps_s[:, 0:w],
                            kT_sb[:, kj * 128:(kj + 1) * 128],
                            qT_sb[:, h, c * CHUNK + off:(c + 1) * CHUNK],
                            start=True, stop=True,
                        )
                        if kj == 1 and norm_pending[0] is not None:
                            emit_norm()
                        if kj >= c * kpc:
                            # causal mask: ps_s column i holds q-position
                            # off+i, so the partial 128-wide diagonal block
                            # is always the first 128 written columns
                            nc.vector.tensor_add(
                                ps_s[:, 0:128], ps_s[:, 0:128], masks_sb,
                            )
                        expT = expt_pool.tile([128, CHUNK], BF16, tag="expT")
                        if off > 0:
                            # exp output is realigned to q-in-chunk columns;
                            # zero the fully-masked leading columns so the
                            # denominator tree can run full-width
                            nc.gpsimd.memset(expT[:, 0:off], 0.0)
                        nc.scalar.activation(
                            expT[:, off:], ps_s[:, 0:w],
                            mybir.ActivationFunctionType.Exp,
                            scale=scale,
                        )
                        pend_pv.append((expT[:, off:], kj, off))
                        if len(pend_pv) > 2:
                            flush_pv()
                        # denominator: bf16 pair tree on DVE (full width --
                        # masked regions of expT are zeroed above)
                        if kj % 2 == 0:
                            stash_exp[0] = expT
                        else:
                            pr = pair_pool.tile([128, CHUNK], BF16, tag="pair")
                            nc.vector.tensor_add(pr, stash_exp[0], expT)
                            stash_exp[0] = None
                            if n_acc[0] == 0 and stash_pair[0] is None:
                                stash_pair[0] = pr
                            elif n_acc[0] == 0:
                                nc.vector.tensor_add(acc, stash_pair[0], pr)
                                stash_pair[0] = None
                                n_acc[0] = 1
                            else:
                                nc.vector.tensor_add(acc, acc, pr)
                                n_acc[0] += 1
                    while pend_pv:
                        flush_pv()
                    # n_kj is always >= 4 so at least two pairs were formed
                    # and acc is initialized by the second pair.
                    assert n_acc[0] >= 1
                    norm_pending[0] = (acc, ps_o, h, c)
                    if c > 0:
                        emit_out_tile(c - 1, h)

            emit_norm()
            for sj in range(spc):
                emit_out_tile(n_ch - 1, sj, last=True)

    return nc


# ---------------------------------------------------------------------------
# Host-side prep


_ROPE_PERM = np.concatenate([np.arange(0, HEAD_DIM, 2), np.arange(1, HEAD_DIM, 2)])


def _prep_tables(freq_cis):
    """RoPE tables in [e, s] permuted-half layout.

    rot[0:64]   = q[0:64]*cos   + q[64:128]*(-sin)
    rot[64:128] = q[64:128]*cos + q[0:64]*sin
    """
    fc = np.asarray(freq_cis, dtype=np.float32)
    A = fc[:, :, 0, 0]    # cos  [S, 64]
    Bm = fc[:, :, 0, 1]   # -sin
    C = fc[:, :, 1, 0]    # sin
    Dm = fc[:, :, 1, 1]   # cos
    t1 = np.concatenate([A, Dm], axis=1).T    # [128, S]
    t2 = np.concatenate([Bm, C], axis=1).T
    return (_bf16(t1), _bf16(t2))


def _prep_masks():
    q = np.arange(128)[None, :]
    p = np.arange(128)[:, None]
    return np.where(q >= p, np.float32(0.0), np.float32(NEG))


def _perm_head_rows(w):
    """Permute rows within each 128-row head block: evens first, odds second."""
    nh = w.shape[0] // HEAD_DIM
    return np.ascontiguousarray(
        w.reshape(nh, HEAD_DIM, -1)[:, _ROPE_PERM, :].reshape(w.shape)
    )


def _bf16(a):
    return np.ascontiguousarray(a.astype(ml_dtypes.bfloat16))


def _pmajor(a):
    """[T*128, F...] -> [128, T, F...] partition-major layout."""
    t = a.shape[0] // 128
    return np.ascontiguousarray(
        a.reshape(t, 128, *a.shape[1:]).swapaxes(0, 1)
    )


def make_core_inputs(x, freq_cis, wq, wk, wv, wo):
    """Build the 8 per-core input maps."""
    x = np.asarray(x, np.float32)
    wq = np.asarray(wq, np.float32)
    wk = np.asarray(wk, np.float32)
    wv = np.asarray(wv, np.float32)
    wo = np.asarray(wo, np.float32)
    t1, t2 = _prep_tables(freq_cis)
    masks = _prep_masks()
    IQ = HQ * HEAD_DIM

    in_maps = []
    for core in range(8):
        b, g = divmod(core, N_GROUPS)
        wq_g = _perm_head_rows(wq[g * IQ:(g + 1) * IQ])
        wk_g = _perm_head_rows(wk[g * HEAD_DIM:(g + 1) * HEAD_DIM])
        wv_g = wv[g * HEAD_DIM:(g + 1) * HEAD_DIM]
        # [D, IQ] -> [128, dt, IQ] -> [128, HQ, dt, HD] head-major
        wqT = _pmajor(_bf16(wq_g.T)).reshape(128, 16, HQ, HEAD_DIM)
        wqT = np.ascontiguousarray(wqT.swapaxes(1, 2))
        wkvT = _pmajor(_bf16(np.concatenate([wk_g.T, wv_g.T], axis=1)))
        woT = _pmajor(_bf16(wo[:, g * IQ:(g + 1) * IQ].T))
        in_maps.append({
            "xT": _pmajor(_bf16(x[b].T)),
            "wqT": wqT,
            "wkvT": wkvT,
            "woT": woT,
            "t1": t1,
            "t2": t2,
            "masks": np.ascontiguousarray(masks),
        })
    return in_maps


_CACHED_NC = None


def _get_nc():
    global _CACHED_NC
    if _CACHED_NC is None:
        from concourse import bacc

        nc = bacc.Bacc("TRN2", target_bir_lowering=False, debug=False)
        build_attention_core(nc)
        nc.compile()
        _CACHED_NC = nc
    return _CACHED_NC


def kernel(x, freq_cis, wq, wk, wv, wo):
    from concourse.bass_utils import run_bass_kernel_spmd

    nc = _get_nc()
    in_maps = make_core_inputs(x, freq_cis, wq, wk, wv, wo)
    res = run_bass_kernel_spmd(nc, in_maps, list(range(8)))
    out = np.zeros((B, S, DIM), dtype=np.float32)
    for core in range(8):
        b = core // N_GROUPS
        out[b] += res.results[core]["out_partial"].astype(np.float32)
    return out


# revision 43
# speedup vs baseline: 1.5201x; 1.0017x over previous
"""Trainium2 Bass kernel for GQA attention forward (B=2, S=2048, D=2048,
16 q-heads / 4 kv-heads, head_dim=128, RoPE, causal).

Sharding: 8 cores = 2 (batch) x 4 (kv-head groups).  Each core computes its
batch's attention for one kv-head group (4 q-heads + 1 kv head) and a
row-parallel partial of the output projection; the host sums the 4 bf16
partials per batch.

Design (all phases software-pipelined over four 512-row s-chunks):
  * q/k projections are emitted directly in [e, s] (transposed) form
    (lhsT = weight tile, rhs = xT tile) -- no PE transposes anywhere.
  * RoPE runs on DVE in the transposed layout via partition-half swaps.
  * Scores keep keys in partitions / queries free, so exp output feeds the
    PV matmul directly.  Only the 128-wide sub-diagonal block is masked
    (columns are realigned so it is always the first written block).
  * The softmax denominator is a bf16 pair tree on DVE over the exp tiles,
    then a 128-partition sum + reciprocal broadcast on the idle Pool
    engine (partition_all_reduce) -- the tensor engine never touches it.
  * Emission order is A(c) | B(*, c) with the previous chunk's output
    projection C(c-1) interleaved after each head of B(c): C's matmuls
    fill the PE gaps where B is exp-throughput-bound, and its PSUM->SBUF
    copies ride whichever of ACT/DVE has slack in that window.
  * x / weights / RoPE tables stream per-chunk (head-major for wq) so the
    first projection chain starts ~2us in; outputs store as bf16 rows.
PSUM budget is exactly 8 banks: k/q/out-proj share 3, scores 3, v/pv 2.
"""

import sys

if "/opt/trn_rl_repo" not in sys.path:
    sys.path.insert(0, "/opt/trn_rl_repo")

import numpy as np
import ml_dtypes

import concourse.bass as bass
import concourse.bass_isa as bass_isa
import concourse.tile as tile
from concourse import mybir

F32 = mybir.dt.float32
F32R = mybir.dt.float32r
BF16 = mybir.dt.bfloat16

# Full-problem constants (per reference).
B, S, DIM = 2, 2048, 2048
N_HEADS, N_KV_HEADS, HEAD_DIM = 16, 4, 128
N_GROUPS = N_KV_HEADS          # tensor-parallel groups
HQ = N_HEADS // N_KV_HEADS     # q heads per group
NEG = -1e30


def build_attention_core(nc, S=S, D=DIM, HQ=HQ, HD=HEAD_DIM, CHUNK=512):
    n_st = S // 128        # s tiles
    n_dt = D // 128        # d tiles
    n_ch = S // CHUNK      # s chunks
    kpc = CHUNK // 128     # k-tiles per chunk
    n_dc = D // CHUNK      # d chunks (phase C)
    spc = CHUNK // 128     # s-tiles per chunk
    IQ = HQ * HD

    x_d = nc.dram_tensor("xT", [128, n_dt, S], BF16, kind="ExternalInput")
    wqT_d = nc.dram_tensor("wqT", [128, HQ, n_dt, HD], BF16,
                           kind="ExternalInput")
    wkvT_d = nc.dram_tensor("wkvT", [128, n_dt, 2 * HD], BF16, kind="ExternalInput")
    woT_d = nc.dram_tensor("woT", [128, IQ // 128, D], BF16, kind="ExternalInput")
    t1_d = nc.dram_tensor("t1", [128, S], BF16, kind="ExternalInput")
    t2_d = nc.dram_tensor("t2", [128, S], BF16, kind="ExternalInput")
    masks_d = nc.dram_tensor("masks", [128, 128], F32, kind="ExternalInput")
    out_d = nc.dram_tensor("out_partial", [S, D], BF16, kind="ExternalOutput")

    scale = float(HD) ** -0.5

    with tile.TileContext(nc) as tc:
        with (
            tc.tile_pool(name="persist", bufs=1) as persist,
            tc.tile_pool(name="xin", bufs=1) as xin_pool,
            tc.tile_pool(name="rope", bufs=4) as rope_pool,
            tc.tile_pool(name="expt", bufs=8) as expt_pool,
            tc.tile_pool(name="acc", bufs=3) as acc_pool,
            tc.tile_pool(name="pairs", bufs=4) as pair_pool,
            tc.tile_pool(name="recip", bufs=3) as rec_pool,
            tc.tile_pool(name="outsb", bufs=4) as outsb_pool,
            # PSUM: 8 banks total
            tc.tile_pool(name="ps_a", bufs=3, space="PSUM") as psa_pool,   # 3
            tc.tile_pool(name="ps_s", bufs=3, space="PSUM") as pss_pool,   # 3
            tc.tile_pool(name="ps_o", bufs=2, space="PSUM") as pso_pool,   # 2
        ):
            # ---------------- weights + constants ---------------------------
            wq_sb = persist.tile([128, HQ, n_dt, HD], BF16)
            wkv_sb = persist.tile([128, n_dt, 2 * HD], BF16)
            t1_sb = persist.tile([128, S], BF16)
            t2_sb = persist.tile([128, S], BF16)
            c0 = slice(0, CHUNK)
            for g in range(n_dt // 4):
                gs = slice(g * 4, (g + 1) * 4)
                nc.scalar.dma_start(out=wkv_sb[:, gs, :], in_=wkvT_d[:, gs, :])
                if g == 0:
                    # chunk-0 RoPE tables early: k-rope needs them ~12us in
                    nc.scalar.dma_start(out=t1_sb[:, c0], in_=t1_d[:, c0])
                    nc.scalar.dma_start(out=t2_sb[:, c0], in_=t2_d[:, c0])
            for h in range(HQ):
                # head-major: q-chain h can start as soon as its slab lands
                nc.scalar.dma_start(out=wq_sb[:, h, :, :], in_=wqT_d[:, h, :, :])
            masks_sb = persist.tile([128, 128], F32)
            nc.scalar.dma_start(out=masks_sb, in_=masks_d[:])
            rest = slice(CHUNK, S)
            nc.scalar.dma_start(out=t1_sb[:, rest], in_=t1_d[:, rest])
            nc.scalar.dma_start(out=t2_sb[:, rest], in_=t2_d[:, rest])
            woT_sb = persist.tile([128, IQ // 128, D], BF16)

            # x streamed per s-chunk on the sync queue
            x_ch = []
            for c in range(n_ch):
                xt = xin_pool.tile([128, n_dt, CHUNK], BF16, tag=f"x{c % 2}",
                                   name=f"x{c}")
                c_sl = slice(c * CHUNK, (c + 1) * CHUNK)
                ng = 2 if c == 0 else 4
                for g4 in range(n_dt // ng):
                    nc.sync.dma_start(
                        out=xt[:, g4 * ng:(g4 + 1) * ng, :],
                        in_=x_d[:, g4 * ng:(g4 + 1) * ng, c_sl],
                    )
                x_ch.append(xt)
                if c == 1:
                    nc.sync.dma_start(out=woT_sb, in_=woT_d[:])

            # persistent activations
            qT_sb = persist.tile([128, HQ, S], BF16)    # [e, h, s]
            kT_sb = persist.tile([128, S], BF16)        # [e, s]
            v_sb = persist.tile([128, n_st, HD], BF16)  # [s_in_tile, s_tile, e]
            oT_sb = persist.tile([128, HQ, S], BF16)    # [e, h, s]

            # deferred per-(h,c) normalization tail (keeps PE from stalling
            # on the DVE denominator chain)
            norm_pending = [None]

            def emit_norm():
                acc_, ps_o_, h_, c_ = norm_pending[0]
                norm_pending[0] = None
                # softmax denominator: 128-partition sum of the bf16 pair
                # tree, broadcast to all partitions, on the idle Pool engine
                sum_sb = rec_pool.tile([128, CHUNK], F32, tag="sum_sb")
                nc.gpsimd.partition_all_reduce(
                    sum_sb, acc_, channels=128, reduce_op=bass_isa.ReduceOp.add
                )
                rec_sb = rec_pool.tile([128, CHUNK], F32, tag="rec_sb")
                nc.vector.reciprocal_approx_fast(rec_sb, sum_sb)
                nc.vector.tensor_mul(
                    oT_sb[:, h_, c_ * CHUNK:(c_ + 1) * CHUNK], ps_o_, rec_sb
                )

            def emit_out_tile(c, sj, last=False):
                """Phase C for s-tile sj of chunk c: one 128-row output slab.
                Interleaved into B(c+1)'s head loop: its matmuls fill the PE
                gaps where B is exp-throughput-bound, and its PSUM->SBUF
                copies run on DVE (ACT is the B-window pacer).  GPSIMD
                cannot access PSUM, so Pool takes no copies."""
                st = c * spc + sj
                row_sb = outsb_pool.tile([128, D], BF16, tag="out_sb")
                for dc in range(n_dc):
                    if last and dc % 2 == 1:
                        ps_d = pss_pool.tile([128, CHUNK], F32, tag="ps_s")
                    else:
                        ps_d = psa_pool.tile([128, CHUNK], F32, tag="ps_a")
                    for it in range(HQ):
                        nc.tensor.matmul(
                            ps_d,
                            oT_sb[:, it, st * 128:(st + 1) * 128],
                            woT_sb[:, it, dc * CHUNK:(dc + 1) * CHUNK],
                            start=(it == 0), stop=(it == HQ - 1),
                        )
                    dst = row_sb[:, dc * CHUNK:(dc + 1) * CHUNK]
                    # engine choice tracks which engine has slack in the
                    # B window this chunk interleaves with (ACT saturates
                    # as c grows; DVE is flatter)
                    if c == 0:
                        use_act = True
                    elif c == 1:
                        use_act = dc % 2 == 0
                    elif c == 2:
                        use_act = False
                    else:
                        use_act = dc % 2 == 0
                    if use_act:
                        nc.scalar.copy(dst, ps_d)
                    else:
                        nc.vector.tensor_copy(dst, ps_d)
                if last and sj == spc - 1:
                    # final tile: split the store so the tail drains as the
                    # copies complete instead of after the whole row
                    for dc in range(n_dc):
                        nc.sync.dma_start(
                            out=out_d[st * 128:(st + 1) * 128,
                                      dc * CHUNK:(dc + 1) * CHUNK],
                            in_=row_sb[:, dc * CHUNK:(dc + 1) * CHUNK],
                        )
                else:
                    nc.sync.dma_start(
                        out=out_d[st * 128:(st + 1) * 128, :], in_=row_sb
                    )

            def rope(dst, src, c):
                """dst[e, s-chunk] = src*t1 + swap_half(src)*t2 (DVE)."""
                c_sl = slice(c * CHUNK, (c + 1) * CHUNK)
                t1c = t1_sb[:, c_sl]
                t2c = t2_sb[:, c_sl]
                m1 = rope_pool.tile([128, CHUNK], F32, tag="m1")
                nc.vector.tensor_mul(m1, src, t1c)
                m2 = rope_pool.tile([128, CHUNK], F32, tag="m2")
                nc.vector.tensor_mul(m2[0:64, :], src[64:128, :], t2c[0:64, :])
                nc.vector.tensor_mul(m2[64:128, :], src[0:64, :], t2c[64:128, :])
                nc.vector.tensor_add(dst, m1, m2)

            for c in range(n_ch):
                c_sl = slice(c * CHUNK, (c + 1) * CHUNK)
                xt = x_ch[c]

                # ======== A(c): projections + RoPE ========
                ps_k = psa_pool.tile([128, CHUNK], F32, tag="ps_a")
                for dt_ in range(n_dt):
                    nc.tensor.matmul(
                        ps_k, wkv_sb[:, dt_, 0:HD], xt[:, dt_, :],
                        start=(dt_ == 0), stop=(dt_ == n_dt - 1),
                    )
                if norm_pending[0] is not None:
                    emit_norm()
                rope(kT_sb[:, c_sl], ps_k, c)

                for h in range(HQ):
                    ps_qh = psa_pool.tile([128, CHUNK], F32, tag="ps_a")
                    for dt_ in range(n_dt):
                        nc.tensor.matmul(
                            ps_qh, wq_sb[:, h, dt_, :], xt[:, dt_, :],
                            start=(dt_ == 0), stop=(dt_ == n_dt - 1),
                        )
                    rope(qT_sb[:, h, c_sl], ps_qh, c)

                # v: natural [s, e] layout, one chain per s-tile; the
                # four chains share one bank from the ps_o rotation
                ps_vt = pso_pool.tile([128, CHUNK], F32, tag="o")
                for sj in range(spc):
                    st = c * spc + sj
                    sj_sl = slice(sj * 128, (sj + 1) * 128)
                    for dt_ in range(n_dt):
                        nc.tensor.matmul(
                            ps_vt[:, sj_sl], xt[:, dt_, sj_sl],
                            wkv_sb[:, dt_, HD:2 * HD],
                            start=(dt_ == 0), stop=(dt_ == n_dt - 1),
                        )
                    nc.scalar.copy(v_sb[:, st, :], ps_vt[:, sj_sl])

                # ======== B(*, c): attention for q-chunk c ========
                for h in range(HQ):
                    ps_o = pso_pool.tile([128, CHUNK], F32, tag="o")
                    n_kj = (c + 1) * kpc
                    acc = acc_pool.tile([128, CHUNK], BF16, tag="acc")
                    pend_pv = []
                    stash_exp = [None]
                    stash_pair = [None]
                    n_acc = [0]

                    def flush_pv():
                        pe, pj, poff = pend_pv.pop(0)
                        nc.tensor.matmul(
                            ps_o[:, poff:], v_sb[:, pj, :], pe,
                            start=(pj == 0), stop=(pj == n_kj - 1),
                        )

                    for kj in range(n_kj):
                        off = max(0, (kj - c * kpc)) * 128
                        w = CHUNK - off
                        ps_s = pss_pool.tile([128, CHUNK], F32, tag="ps_s")
                        nc.tensor.matmul(
                            ps_s[:, 0:w],
                            kT_sb[:, kj * 128:(kj + 1) * 128],
                            qT_sb[:, h, c * CHUNK + off:(c + 1) * CHUNK],
                            start=True, stop=True,
                        )
                        if kj == 1 and norm_pending[0] is not None:
                            emit_norm()
                        if kj >= c * kpc:
                            # causal mask: ps_s column i holds q-position
                            # off+i, so the partial 128-wide diagonal block
                            # is always the first 128 written columns
                            nc.vector.tensor_add(
                                ps_s[:, 0:128], ps_s[:, 0:128], masks_sb,
                            )
                        expT = expt_pool.tile([128, CHUNK], BF16, tag="expT")
                        if off > 0:
                            # exp output is realigned to q-in-chunk columns;
                            # zero the fully-masked leading columns so the
                            # denominator tree can run full-width
                            nc.gpsimd.memset(expT[:, 0:off], 0.0)
                        nc.scalar.activation(
                            expT[:, off:], ps_s[:, 0:w],
                            mybir.ActivationFunctionType.Exp,
                            scale=scale,
                        )
                        pend_pv.append((expT[:, off:], kj, off))
                        if len(pend_pv) > 2:
                            flush_pv()
                        # denominator: bf16 pair tree on DVE (full width --
                        # masked regions of expT are zeroed above)
                        if kj % 2 == 0:
                            stash_exp[0] = expT
                        else:
                            pr = pair_pool.tile([128, CHUNK], BF16, tag="pair")
                            nc.vector.tensor_add(pr, stash_exp[0], expT)
                            stash_exp[0] = None
                            if n_acc[0] == 0 and stash_pair[0] is None:
                                stash_pair[0] = pr
                            elif n_acc[0] == 0:
                                nc.vector.tensor_add(acc, stash_pair[0], pr)
                                stash_pair[0] = None
                                n_acc[0] = 1
                            else:
                                nc.vector.tensor_add(acc, acc, pr)
                                n_acc[0] += 1
                    while pend_pv:
                        flush_pv()
                    # n_kj is always >= 4 so at least two pairs were formed
                    # and acc is initialized by the second pair.
                    assert n_acc[0] >= 1
                    norm_pending[0] = (acc, ps_o, h, c)
                    if c > 0:
                        emit_out_tile(c - 1, h)

            emit_norm()
            for sj in range(spc):
                emit_out_tile(n_ch - 1, sj, last=True)

    return nc


# ---------------------------------------------------------------------------
# Host-side prep


_ROPE_PERM = np.concatenate([np.arange(0, HEAD_DIM, 2), np.arange(1, HEAD_DIM, 2)])


def _prep_tables(freq_cis):
    """RoPE tables in [e, s] permuted-half layout.

    rot[0:64]   = q[0:64]*cos   + q[64:128]*(-sin)
    rot[64:128] = q[64:128]*cos + q[0:64]*sin
    """
    fc = np.asarray(freq_cis, dtype=np.float32)
    A = fc[:, :, 0, 0]    # cos  [S, 64]
    Bm = fc[:, :, 0, 1]   # -sin
    C = fc[:, :, 1, 0]    # sin
    Dm = fc[:, :, 1, 1]   # cos
    t1 = np.concatenate([A, Dm], axis=1).T    # [128, S]
    t2 = np.concatenate([Bm, C], axis=1).T
    return (_bf16(t1), _bf16(t2))


def _prep_masks():
    q = np.arange(128)[None, :]
    p = np.arange(128)[:, None]
    return np.where(q >= p, np.float32(0.0), np.float32(NEG))


def _perm_head_rows(w):
    """Permute rows within each 128-row head block: evens first, odds second."""
    nh = w.shape[0] // HEAD_DIM
    return np.ascontiguousarray(
        w.reshape(nh, HEAD_DIM, -1)[:, _ROPE_PERM, :].reshape(w.shape)
    )


def _bf16(a):
    return np.ascontiguousarray(a.astype(ml_dtypes.bfloat16))


def _pmajor(a):
    """[T*128, F...] -> [128, T, F...] partition-major layout."""
    t = a.shape[0] // 128
    return np.ascontiguousarray(
        a.reshape(t, 128, *a.shape[1:]).swapaxes(0, 1)
    )


def make_core_inputs(x, freq_cis, wq, wk, wv, wo):
    """Build the 8 per-core input maps."""
    x = np.asarray(x, np.float32)
    wq = np.asarray(wq, np.float32)
    wk = np.asarray(wk, np.float32)
    wv = np.asarray(wv, np.float32)
    wo = np.asarray(wo, np.float32)
    t1, t2 = _prep_tables(freq_cis)
    masks = _prep_masks()
    IQ = HQ * HEAD_DIM

    in_maps = []
    for core in range(8):
        b, g = divmod(core, N_GROUPS)
        wq_g = _perm_head_rows(wq[g * IQ:(g + 1) * IQ])
        wk_g = _perm_head_rows(wk[g * HEAD_DIM:(g + 1) * HEAD_DIM])
        wv_g = wv[g * HEAD_DIM:(g + 1) * HEAD_DIM]
        # [D, IQ] -> [128, dt, IQ] -> [128, HQ, dt, HD] head-major
        wqT = _pmajor(_bf16(wq_g.T)).reshape(128, 16, HQ, HEAD_DIM)
        wqT = np.ascontiguousarray(wqT.swapaxes(1, 2))
        wkvT = _pmajor(_bf16(np.concatenate([wk_g.T, wv_g.T], axis=1)))
        woT = _pmajor(_bf16(wo[:, g * IQ:(g + 1) * IQ].T))
        in_maps.append({
            "xT": _pmajor(_bf16(x[b].T)),
            "wqT": wqT,
            "wkvT": wkvT,
            "woT": woT,
            "t1": t1,
            "t2": t2,
            "masks": np.ascontiguousarray(masks),
        })
    return in_maps


_CACHED_NC = None


def _get_nc():
    global _CACHED_NC
    if _CACHED_NC is None:
        from concourse import bacc

        nc = bacc.Bacc("TRN2", target_bir_lowering=False, debug=False)
        build_attention_core(nc)
        nc.compile()
        _CACHED_NC = nc
    return _CACHED_NC


def kernel(x, freq_cis, wq, wk, wv, wo):
    from concourse.bass_utils import run_bass_kernel_spmd

    nc = _get_nc()
    in_maps = make_core_inputs(x, freq_cis, wq, wk, wv, wo)
    res = run_bass_kernel_spmd(nc, in_maps, list(range(8)))
    out = np.zeros((B, S, DIM), dtype=np.float32)
    for core in range(8):
        b = core // N_GROUPS
        out[b] += res.results[core]["out_partial"].astype(np.float32)
    return out
